# revision 1
# baseline (speedup 1.0000x reference)
"""Differentiable palette quantization on 8 Trainium2 NeuronCores.

Math: for each image b, pixel x, palette p_k (k=64):
    w = softmax_k(-|x - p_k|^2 / T);  out = sum_k w_k p_k
Softmax is invariant to the per-pixel |x|^2 term, so the logit reduces to
    (2/T)*dot(x, p_k) + bias_k  with exact bias_k = -|p_k|^2/T supplied
through the ACT activation's per-partition bias operand.  The weighted
sum and the softmax denominator come from one matmul against
[palette | ones] (contraction over k = partitions).

Sharding: pure data parallel, 2 images per core.  A core's two images are
stacked on partitions (64+64 palette entries) and share the pixel stream
via a block-diagonal stationary matrix.  The dot matmul runs in bf16
(rel err ~1.3e-3, tolerance 2e-2; the exact hi/lo fixup is available via
PALQ_MM1_SPLIT=1 at 3x the input-DMA cost).

ACT (exp) is the bottleneck engine: 65536 psum columns/core at
0.833ns/col ~= 54.6us floor.  The schedule keeps ACT gapless and
minimizes head (time to first exp) and tail (after last exp):
 - round 0 and the final round carry a single 512-col matmul task so the
   first exp waits on one matmul only and the tail epilogue is short
   (tile-granular dependency tracking makes sub-tile splits useless);
 - middle rounds pack 3 tasks into a [128,1536] psum tile (3 banks x 2
   buffers) for one FD=1536 exp op each;
 - ebias rides the gpsimd SWDGE queue, palt + strips 2/3 ride the scalar
   HWDGE queue, strips 0/1 the sync queue, so all head DMAs generate
   descriptors concurrently;
 - two tiny warm-up matmuls at t~7us start the PE p-state clock ramp
   before the first real matmul.

The e values are written as fp16 so the weighted-sum matmuls (lhsT =
e-block [128, 128], rhs = palW [128, 8]) get fast weight load.  Their
[128, 8] outputs accumulate in psum2 allocations of 60 subtiles (1 bank
x 2 buffers); the last 32 subtiles split into 16/8/4/4-subtile
allocations so each flush depends only on its own matmuls.  Flushes:
DVE computes numer * 1/denom into one [128, 6*nu] tile and a single DMA
ships both images' columns (early blocks on the gpsimd SWDGE queue,
late ones on the vector/sync HWDGE queues -- never the scalar queue,
whose sequencer is the ACT engine running the exp stream).
"""

import os
import sys

for _p in ("/opt/trn_rl_repo", os.path.expanduser("~/.axon_site/_ro/trn_rl_repo")):
    if os.path.isdir(_p) and _p not in sys.path:
        sys.path.insert(0, _p)

# After sustained benchmarking this device can latch a ~20%-slower clock
# state (identical program: 79us -> 94us; every engine uniformly 1.2x).
# A core reset at runtime init restores full clocks, and costs nothing
# when clocks are already normal (measured 79805ns with this set).
os.environ.setdefault("NEURON_RT_RESET_CORES", "1")

import numpy as np

import concourse.bass as bass
import concourse.tile as tile
from concourse import bacc, mybir
from concourse.bass_utils import run_bass_kernel_spmd

# problem constants (hardcoded per contract)
B, H, W, C, K = 16, 256, 256, 3, 64
NCORES = 8
IMGS_PER_CORE = B // NCORES            # 2
P = H * W                              # 65536 pixel-pairs per core
NQ = 4                                 # PE row-tile quarters
QP = P // NQ                           # 16384 pixels per quarter
RN = 512                               # pixels per strip-task
NTASKS = NQ * (QP // RN)               # 128 matmul tasks
NSUB = NTASKS * 4                      # 512 subtiles of 128 pixels
# psum2 allocation sizes in subtiles; last block split small so the tail
# flush depends only on the final few matmuls
FLUSH_SIZES = [60] * 8 + [20, 8, 4]
assert sum(FLUSH_SIZES) == NSUB

# tuning knobs (env-overridable for experiments)
MM1_DT = os.environ.get("PALQ_MM1_DT", "bfloat16")   # bfloat16|float32
E_DT = os.environ.get("PALQ_E_DT", "float16")        # float16|float32|bfloat16
MM1_SPLIT = os.environ.get("PALQ_MM1_SPLIT", "0") == "1"    # hi/lo x & palT
WARM_PE = os.environ.get("PALQ_WARM_PE", "1") == "1"
# exp-offload width: the last OW of each 3-task round's 1536 psum columns
# skip the ACT engine and go through DVE (affine f32->f16 from PSUM, then
# fp16 poly exp).  GPSIMD cannot touch PSUM, so DVE does the copy too.
# Measured: DVE costs ~3.9ns/col (1x f32 affine + 2x-mode muls + per-op
# overhead) vs ACT's 0.83, and the psum-recycle coupling adds jitter --
# every tested width (128/256/384) lost to OW=0, so the offload stays
# disabled; the machinery is kept for reference.  Multiple of 128.
OW = int(os.environ.get("PALQ_OW", "0"))
OBATCH = 4                                   # offload rounds per DVE pass

# degree-3 minimax-ish fit of e^u on u in [-0.75, 0.85] (u = logit/4);
# two fp16 squarings recover e^logit.  End-to-end rel err ~2e-3.
# The poly is evaluated MONIC (divided by a3): the uniform a3^4 factor on
# e cancels between the weighted sum and the softmax denominator.  Ops
# are chosen by measured DVE speed: tensor_scalar adds run at 4x,
# tensor_tensor muls at 2x, scalar_tensor_tensor only at 1x.
EXP_A3, EXP_A2, EXP_A1, EXP_A0 = (0.1673257, 0.52268187, 1.00206876,
                                  0.9987321)
EXP_H2 = EXP_A2 / EXP_A3
EXP_H1 = EXP_A1 / EXP_A3
EXP_C0 = EXP_A0 / EXP_A3


def _dt(name):
    return getattr(mybir.dt, name)


def _round_plan():
    """Rounds as lists of (task, q0, nq) 128-px-quarter segments.

    Round 0 waits on one matmul only; round 1 on two (whose strips
    arrive on separate DMA queues), so the exp stream ramps with the
    head DMAs.  (A 128-col mini round 0 with its own sliver DMA starts
    the stream 0.2us earlier but ends it at the same time — the stream
    is gated by strips 1-3's chunk-0 DMA latency, not strip 0's.)  The
    final single-task round keeps the tail epilogue short.
    """
    rounds = [[(0, 0, 4)], [(1, 0, 4), (2, 0, 4)]]
    t = 3
    while t + 3 <= NTASKS - 2:
        rounds.append([(t, 0, 4), (t + 1, 0, 4), (t + 2, 0, 4)])
        t += 3
    while t < NTASKS:
        rounds.append([(t, 0, 4)])
        t += 1
    return rounds


def _schedule():
    """Emission plan shared by the device loop and the host decode.

    Returns (rounds, plan) where plan[r] = (act_subs, batch_close):
    act_subs = [(task, quarter), ...] exp'd on ACT this round;
    batch_close = None, or the list of rounds in the offload batch that
    DVE finishes after round r (their offload subtiles then hit mm2 in
    round order, [(task, quarter), ...] each).

    A psum column-subtile t (128 px) of a round maps to task tasks[t//4],
    quarter t%4.  The offload covers the LAST OW/128 subtiles of 3-task
    rounds (except the first 3-task round, which stays pure ACT so the
    head DMA stream isn't raced by Pool work).
    """
    rounds = _round_plan()
    noff = OW // 128
    off_rounds = [r for r, t in enumerate(rounds) if len(t) == 3][1:] \
        if OW else []
    plan = []
    pend = []
    for r, segs in enumerate(rounds):
        subs = [(i, q0 + q) for (i, q0, nq) in segs for q in range(nq)]
        if r in off_rounds:
            act_subs, off_subs = subs[:12 - noff], subs[12 - noff:]
            pend.append((r, off_subs))
            close = None
            if len(pend) == OBATCH or r == off_rounds[-1]:
                close = pend
                pend = []
            plan.append((act_subs, close))
        else:
            plan.append((subs, None))
    return rounds, plan


def build_bass(scale: float):
    nc = bacc.Bacc("TRN2", target_bir_lowering=False, debug=False)
    f32 = mybir.dt.float32
    e_dt = _dt(E_DT)
    mm1_dt = _dt(MM1_DT)

    # contraction rows per quarter: 6 = (rgb x 2 images); with hi/lo
    # split, 18 = [xh | xl | xh] against [ph | ph | pl]
    kr = 18 if MM1_SPLIT else 6
    xin = nc.dram_tensor("xin", [NQ, kr, QP], mm1_dt, kind="ExternalInput")
    palt = nc.dram_tensor("palt", [128, 128], mm1_dt, kind="ExternalInput")
    # col 0: exact -|p|^2 * scale/2 softmax bias; col 1: same / 4 (for the
    # quarter-logit offload path)
    ebias2 = nc.dram_tensor("ebias2", [128, 2], f32, kind="ExternalInput")
    palw_hi = nc.dram_tensor("palw_hi", [128, 8], e_dt, kind="ExternalInput")
    out = nc.dram_tensor("out", [IMGS_PER_CORE, 128, 3 * NSUB], f32,
                         kind="ExternalOutput")

    rounds, plan = _schedule()

    with tile.TileContext(nc) as tc:
        import contextlib
        with contextlib.ExitStack() as ctx:
            singles = ctx.enter_context(tc.tile_pool(name="singles", bufs=1))
            epool = ctx.enter_context(tc.tile_pool(name="epool", bufs=4))
            # PSUM: 2x3 banks psum1 + 2x1 banks psum2 = all 8 banks.
            # (Tested alternatives that lost: 4+3+1 asymmetric tiles
            # with single-buffered psum2 — fewer ACT ops but ~0.4us
            # mm2-vs-DVE WAR stalls per block; interleaved bank
            # placement — the ~90ns/op ACT duration creep is a
            # saturation ramp of the gapless exp stream, not bank-group
            # contention, so moving banks changes nothing.)
            ps1 = ctx.enter_context(tc.tile_pool(name="ps1", bufs=2, space="PSUM"))
            ps2 = ctx.enter_context(tc.tile_pool(name="ps2", bufs=2, space="PSUM"))
            vpool = ctx.enter_context(tc.tile_pool(name="vpool", bufs=2))
            opool = ctx.enter_context(tc.tile_pool(name="opool", bufs=3))
            if OW:
                ubpool = ctx.enter_context(tc.tile_pool(name="ubpool", bufs=2))
                ypool = ctx.enter_context(tc.tile_pool(name="ypool", bufs=2))
                eoffp = ctx.enter_context(tc.tile_pool(name="eoffp", bufs=2))

            # stationary palette on the scalar HWDGE queue
            palt_sb = singles.tile([128, 128], mm1_dt)
            nc.scalar.dma_start(out=palt_sb, in_=palt.ap())

            # pre-warm the ACT exp table while input DMAs stream
            warm = singles.tile([1, 1], f32)
            nc.scalar.activation(out=warm,
                                 in_=nc.const_aps.scalar_like(0.0, warm),
                                 func=mybir.ActivationFunctionType.Exp)

            # tiny SBUF source for the PE p-state warm-up matmuls
            if WARM_PE:
                warm_pe = singles.tile([2, 2], mm1_dt)
                nc.gpsimd.memset(warm_pe, 0.0)

            # resident input pixels: quarter j on partitions [32j, 32j+kr)
            xsb = singles.tile([128, QP], mm1_dt)
            ebias_sb = singles.tile([128, 2], f32)
            # tiny, needed by the first exp: SWDGE so it skips the busy
            # HWDGE queues entirely (tested on scalar after palt: lands
            # earlier but delays strip 2 and measured slightly worse)
            nc.gpsimd.dma_start(out=ebias_sb, in_=ebias2.ap())
            palw_sb = singles.tile([128, 8], e_dt)
            bounds = [0, 512, 1536, 2560, 4096, 6144, 8192, 10240, 12288,
                      14336, QP]
            # chunk 0 of the four strips fans out over four queues so
            # rounds 0 and 1 are fed as early as possible.  Everything
            # else rides the sync queue: its SP sequencer is otherwise
            # idle, and crucially the scalar queue's sequencer IS the
            # ACT engine -- any scalar-queue DMA issued after the exp
            # stream starts stalls the bottleneck engine ~700ns.
            # strips 0+1 on sync (strip 1's matmuls gate round 1 -- the
            # gpsimd SWDGE's ~1us fixed latency behind the ebias DMA
            # made it the round-1 critical path); strip 2 on scalar,
            # strip 3 on gpsimd (not needed until round 2)
            chunk0 = {0: nc.sync, 1: nc.sync, 2: nc.scalar, 3: nc.gpsimd}
            for h in range(len(bounds) - 1):
                for j in range(NQ):
                    sl = slice(bounds[h], bounds[h + 1])
                    eng = chunk0[j] if h == 0 else nc.sync
                    eng.dma_start(out=xsb[32 * j:32 * j + kr, sl],
                                  in_=xin.ap()[j, :, sl])
                if h == 0:
                    # needed by the first weighted-sum matmuls (~12us in)
                    nc.scalar.dma_start(out=palw_sb, in_=palw_hi.ap())

            # main stream: per round, mm1 tasks -> one exp -> mm2 subtiles
            s = 0               # global 128-pixel subtile counter
            blk = 0             # psum2 allocation index
            blk_tile = None
            blk_s0 = 0

            def flush(tile_, s0, nu, eng):
                # 3 DVE ops per flush: one dual reciprocal (both images'
                # denominators via the stride-4 column pair) and one
                # broadcast multiply per image.  Small-FD DVE ops cost
                # ~150ns of PSUM-access latency each, so op count — not
                # element count — is what the tail pays for.
                psr = tile_.rearrange("p (v e) -> p v e", e=8)
                rec = vpool.tile([128, nu, 2], f32, name="rec")
                nc.vector.reciprocal(out=rec, in_=psr[:, :, 3:8:4])
                outAB = opool.tile([128, 6 * nu], f32, name="outAB")
                for img in range(2):
                    o3 = outAB[:, 3 * nu * img:3 * nu * (img + 1)].rearrange(
                        "p (v c) -> p v c", c=3)
                    nc.vector.tensor_mul(
                        out=o3, in0=psr[:, :, 4 * img:4 * img + 3],
                        in1=rec[:, :, img:img + 1].broadcast_to(o3.shape))
                # one DMA for both images: dst [128, 2, 3nu] (partition-
                # major view of out), src [128, (img, c)] tile
                dst = out.ap().rearrange("i p c -> p i c")[:, :, 3 * s0:3 * (s0 + nu)]
                src = outAB.rearrange("p (i c) -> p i c", i=2)
                eng.dma_start(out=dst, in_=src)

            def mm2(esrc, c0):
                nonlocal s, blk, blk_tile, blk_s0
                if blk_tile is None:
                    blk_tile = ps2.tile([128, 8 * FLUSH_SIZES[blk]], f32,
                                        name="psum2")
                    blk_s0 = s
                u = s - blk_s0
                nc.tensor.matmul(
                    out=blk_tile[:, 8 * u:8 * u + 8],
                    lhsT=esrc[:, c0:c0 + 128],
                    rhs=palw_sb,
                    start=True, stop=True,
                )
                s += 1
                if s - blk_s0 == FLUSH_SIZES[blk]:
                    # early blocks ride the gpsimd SWDGE (idle-ish Pool
                    # engine); late ones the sync HWDGE, so no SWDGE
                    # transfer is pending at kernel end (that costs a
                    # ~2.6us drain).  The scalar queue's sequencer is
                    # the ACT engine, so it must carry nothing while
                    # exps run -- but the LAST flush is emitted after
                    # the final exp, so the now-idle ACT sequencer can
                    # generate it immediately instead of queueing it
                    # behind the sync queue's earlier flush DMAs.
                    if blk == len(FLUSH_SIZES) - 1:
                        eng = nc.scalar
                    else:
                        eng = nc.gpsimd if blk <= 6 else nc.sync
                    flush(blk_tile, blk_s0, FLUSH_SIZES[blk], eng)
                    blk += 1
                    blk_tile = None

            ub = None
            ub_fill = 0
            for r, segs in enumerate(rounds):
                nt = len(segs)
                act_subs, close = plan[r]
                na = len(act_subs)
                psum1 = ps1.tile([128, 3 * RN], f32)
                if r == 0 and WARM_PE:
                    for _ in range(2):
                        nc.tensor.matmul(out=psum1[0:2, 0:2], lhsT=warm_pe,
                                         rhs=warm_pe, start=True, stop=True)
                col = 0
                for (i, q0, nq) in segs:
                    j, k = i % NQ, i // NQ
                    psl = slice(32 * j, 32 * j + kr)
                    x0 = RN * k + 128 * q0
                    nc.tensor.matmul(
                        out=psum1[:, col:col + 128 * nq],
                        lhsT=palt_sb[psl, :],
                        rhs=xsb[psl, x0:x0 + 128 * nq],
                        start=True, stop=True,
                        tile_position=(32 * j, 0),
                    )
                    col += 128 * nq
                e_sb = epool.tile([128, 128 * na], e_dt)
                nc.scalar.activation(
                    out=e_sb, in_=psum1[:, 0:128 * na],
                    func=mybir.ActivationFunctionType.Exp,
                    scale=float(scale), bias=ebias_sb[:, 0:1],
                )
                if na < sum(nq for (_, _, nq) in segs):
                    # offload columns: DVE applies scale/4 and bias/4,
                    # writing quarter-logits u as fp16 into the batch tile
                    # (GPSIMD can't read PSUM; this one op runs at f32
                    # speed, the rest of the chain at fp16 speed)
                    if ub is None:
                        ub = ubpool.tile([128, OW * OBATCH], e_dt)
                        ub_fill = 0
                    nc.vector.tensor_scalar(
                        out=ub[:, OW * ub_fill:OW * (ub_fill + 1)],
                        in0=psum1[:, 128 * na:3 * RN],
                        scalar1=float(scale) * 0.25,
                        scalar2=ebias_sb[:, 1:2],
                        op0=mybir.AluOpType.mult, op1=mybir.AluOpType.add)
                    ub_fill += 1
                for t in range(na):
                    mm2(e_sb, 128 * t)
                if close is not None:
                    # DVE: y = ((u+h2)u + h1)u + c0 (monic cubic ~ e^u/a3),
                    # then e = y^4.  TS adds at 4x, TT muls at 2x.
                    bw = OW * len(close)
                    ubv = ub[:, 0:bw]
                    y = ypool.tile([128, OW * OBATCH], e_dt, name="y")
                    yv = y[:, 0:bw]
                    nc.vector.tensor_scalar_add(out=yv, in0=ubv,
                                                scalar1=float(EXP_H2))
                    nc.vector.tensor_mul(out=yv, in0=yv, in1=ubv)
                    nc.vector.tensor_scalar_add(out=yv, in0=yv,
                                                scalar1=float(EXP_H1))
                    nc.vector.tensor_mul(out=yv, in0=yv, in1=ubv)
                    e_off = eoffp.tile([128, OW * OBATCH], e_dt, name="e_off")
                    ev = e_off[:, 0:bw]
                    nc.vector.tensor_scalar_add(out=ev, in0=yv,
                                                scalar1=float(EXP_C0))
                    nc.vector.tensor_mul(out=ev, in0=ev, in1=ev)
                    nc.vector.tensor_mul(out=ev, in0=ev, in1=ev)
                    w = 0
                    for _, off_subs in close:
                        for _ in off_subs:
                            mm2(e_off, 128 * w)
                            w += 1
                    ub = None

    nc.compile()
    return nc


def _host_prep(images, palettes, scale):
    """Per-core input arrays. images [16,256,256,3] f32, palettes [16,64,3].
    scale = 2/temperature; the softmax logit is scale*dot + ebias."""
    import ml_dtypes

    imgs = np.ascontiguousarray(images, np.float32).reshape(B, P, C)
    pals = np.ascontiguousarray(palettes, np.float32)
    np_mm1 = {"float16": np.float16,
              "bfloat16": ml_dtypes.bfloat16}.get(MM1_DT, np.float32)
    np_e = {"float16": np.float16,
            "bfloat16": ml_dtypes.bfloat16}.get(E_DT, np.float32)
    in_maps = []
    for core in range(NCORES):
        ia, ib = imgs[2 * core], imgs[2 * core + 1]
        # per-quarter channel rows: [rgbA | rgbB] on the contraction dim
        x6 = np.empty((NQ, 6, QP), np.float32)
        x6[:, 0:3] = ia.reshape(NQ, QP, C).transpose(0, 2, 1)
        x6[:, 3:6] = ib.reshape(NQ, QP, C).transpose(0, 2, 1)

        pa, pb = pals[2 * core], pals[2 * core + 1]
        p6 = np.zeros((6, 128), np.float32)   # block-diag [pA^T | pB^T]
        p6[0:3, 0:64] = pa.T
        p6[3:6, 64:128] = pb.T

        if MM1_SPLIT:
            xh = x6.astype(np_mm1)
            xl = (x6 - xh.astype(np.float32)).astype(np_mm1)
            ph = p6.astype(np_mm1)
            pl = (p6 - ph.astype(np.float32)).astype(np_mm1)
            xin = np.concatenate([xh, xl, xh], axis=1)       # [NQ, 18, QP]
            palt_kr = np.concatenate([ph, ph, pl], axis=0)   # [18, 128]
        else:
            xin = x6.astype(np_mm1)
            palt_kr = p6.astype(np_mm1)
        kr = palt_kr.shape[0]
        palt = np.zeros((128, 128), palt_kr.dtype)           # strip-replicated
        for j in range(NQ):
            palt[32 * j:32 * j + kr] = palt_kr

        ebias2 = np.empty((128, 2), np.float32)
        ebias2[0:64, 0] = -0.5 * scale * (pa * pa).sum(-1)
        ebias2[64:128, 0] = -0.5 * scale * (pb * pb).sum(-1)
        ebias2[:, 1] = 0.25 * ebias2[:, 0]

        palw = np.zeros((128, 8), np.float32)
        palw[0:64, 0:3] = pa
        palw[0:64, 3] = 1.0
        palw[64:128, 4:7] = pb
        palw[64:128, 7] = 1.0

        m = {"xin": xin, "palt": palt, "ebias2": ebias2,
             "palw_hi": palw.astype(np_e)}
        in_maps.append(m)
    return in_maps


def _subtile_base():
    """Pixel base offset for each global subtile s, mirroring the device
    emission order from _schedule(): per round the ACT subtiles, and
    after each offload-batch close, that batch's offload subtiles."""
    _, plan = _schedule()
    seq = []
    for act_subs, close in plan:
        seq.extend(act_subs)
        if close is not None:
            for _, off_subs in close:
                seq.extend(off_subs)
    assert len(seq) == NSUB
    base = np.empty(NSUB, np.int64)
    for s, (i, q) in enumerate(seq):
        base[s] = (i % NQ) * QP + (i // NQ) * RN + q * 128
    return base


def _host_post(results):
    """results[core]["out"] [2, 128, 1536] -> [16, 256, 256, 3]."""
    base = _subtile_base()
    out = np.empty((B, P, C), np.float32)
    for core in range(NCORES):
        o = results[core]["out"]
        dec = np.empty((IMGS_PER_CORE, P, C), np.float32)
        for s in range(NSUB):
            dec[:, base[s]:base[s] + 128, :] = o[:, :, 3 * s:3 * s + 3]
        out[2 * core] = dec[0]
        out[2 * core + 1] = dec[1]
    return out.reshape(B, H, W, C)


_CACHE = {}


def _get_nc(scale: float):
    key = (round(float(scale), 12), MM1_DT, E_DT, MM1_SPLIT, WARM_PE, OW)
    if key not in _CACHE:
        _CACHE[key] = build_bass(scale)
    return _CACHE[key]


def kernel(images, palettes, temperature, _trace=False):
    scale = 2.0 / float(np.asarray(temperature))
    nc = _get_nc(scale)
    in_maps = _host_prep(images, palettes, scale)
    res = run_bass_kernel_spmd(nc, in_maps, core_ids=list(range(NCORES)),
                               trace=_trace)
    out = _host_post(res.results)
    if _trace:
        kernel.last_result = res
    return out

